# revision 30
# baseline (speedup 1.0000x reference)
"""Trainium2 Bass kernel for nn_BinaryDiff: out = x @ base + coeff * (x @ mask).

Fused as a single matmul: out = x @ W where W = base + coeff * mask.

Sharding over 8 NeuronCores: data-parallel over rows — each core computes
1024 rows x 4096 cols of the [8192, 4096] output (x sharded by rows, W
replicated). x is pre-transposed/cast on the host as part of the
sharding/layout prep, so the device program is a pure matmul pipeline.

Mixed precision: 28 of 32 k-slabs run bf16; k-slabs 0-3 run fp8(e4m3)
with perf_mode=DoubleRow (two k-slabs per matmul, ~2x throughput), which
keeps the measured rel-err ~1.4e-2 (sim-verified on the fixed inputs)
under the 2e-2 gate while cutting ~6% of PE time. Both paths share one
power-of-2 pre-scaling (x*16, W*1024 — keeps fp8 operands out of the
subnormal range) so they accumulate into the same PSUM bank at scale
2^14; the PSUM drain copies descale by 2^-14. The fp8 pairs are placed
LAST in every panel's accumulation so their operands have the most
relaxed arrival deadlines.

Pipeline structure:
  - x^T shard resident in SBUF, slab DMAs split across the SWDGE and SP
    rings (a single queue only gets ~1/3 of HBM bandwidth under
    round-robin) with the first slabs racing ahead to build run-ahead.
  - W streamed in eight 512-col panels (p-major host layout, contiguous
    per-partition lines), base chunks on the SP HWDGE ring, mask chunks
    on the ACT HWDGE ring, fused W = base*1024 + (c*1024)*mask on DVE
    into double-buffered panel caches (bf16 + fp8), one panel of
    prefetch lead. Panel np+2's chunk DMAs are anchored on panel np's
    last drain copy so the scheduler cannot hoist them ahead of the
    drains in the engine queues (that head-of-line-blocks the copies,
    pins PSUM banks, and stalls the next panel). W1 rides the ACT ring
    dep-paced into panel 0's middle, and panel 1 accumulates kt in
    REVERSE so its deadlines match W1's late high-kt arrivals.
  - Warm-up matmuls on a memset tile while the first chunks land, so the
    PE HAM clock-gate is already 8/8 when real work starts.
  - Panel 0 runs as one 8-bank wave (matmul consumption matches DMA
    delivery); later panels run 4-bank waves rotating through the 8 PSUM
    banks (drains overlap the next wave, no bank-handoff stalls); the
    last panel drains 4/2/1/1 with the final DMAs split across both
    HWDGE rings to shorten the end-of-kernel tail.
  - Panel-0 drains are ACT-only (a DVE copy would sit behind W1's fuses
    in the strict-FIFO DVE queue); later panels alternate ScalarE/DVE.
"""

import numpy as np
import ml_dtypes

import concourse.mybir as mybir
import concourse.tile as tile
from concourse import bacc

P = 128
FULL_M, FULL_K, FULL_N = 8192, 4096, 4096
N_CORES = 8
CORE_M = FULL_M // N_CORES      # 1024 rows per core
K_T = FULL_K // P               # 32 k-slabs
M_T = CORE_M // P               # 8 m-tiles
NPAN = 8                        # W panels across N
N_MM = FULL_N // NPAN           # 512 (one PSUM bank)
N_WARM = 26                     # HAM warm-up matmuls
# F8=6 with the 3 DoubleRow pairs back-to-back trips the chip's P0 power
# throttle (PE 2.4 -> 2.0 GHz for the whole run, +15% wall) — the fp8
# DoubleRow cells draw ~2x power. Spreading the pairs through the bf16
# sequence (panels 2+) keeps the burst short.
F8 = 6                          # leading k-slabs in fp8 DoubleRow
NPAIR = F8 // 2                 # DoubleRow pairs
SCALE_X = 16.0                  # fp8 subnormal-avoidance pre-scales
SCALE_W = 1024.0
DESCALE = 1.0 / (SCALE_X * SCALE_W)


def build_kernel(debug=False):
    """Build the per-core Bass program. All cores run the same program (SPMD)."""
    f32 = mybir.dt.float32
    i8 = mybir.dt.int8
    bf16 = mybir.dt.bfloat16
    f8 = mybir.dt.float8e4

    nc = bacc.Bacc("TRN2", target_bir_lowering=False, debug=debug)

    xT_d = nc.dram_tensor("xT", [FULL_K, CORE_M], bf16, kind="ExternalInput").ap()
    xT8_d = nc.dram_tensor("xT8", [F8 * P, CORE_M], f8, kind="ExternalInput").ap()
    # base/mask pretiled on host to [NPAN, P, K_T, N_MM] (p-major panels,
    # flattened to 2D) so per-partition DMA lines are contiguous.
    base_d = nc.dram_tensor(
        "baseT", [NPAN * P, K_T * N_MM], bf16, kind="ExternalInput"
    ).ap()
    mask_d = nc.dram_tensor(
        "maskT", [NPAN * P, K_T * N_MM], i8, kind="ExternalInput"
    ).ap()
    coeff_d = nc.dram_tensor("coeff", [P, 1], f32, kind="ExternalInput").ap()
    out_d = nc.dram_tensor("out", [CORE_M, FULL_N], f32, kind="ExternalOutput").ap()

    with tile.TileContext(nc) as tc:
        with (
            tc.tile_pool(name="const", bufs=1) as const,
            tc.tile_pool(name="xts", bufs=1) as xpool,
            tc.tile_pool(name="bstage", bufs=4) as bstage,
            tc.tile_pool(name="mstage", bufs=4) as mstage,
            tc.tile_pool(name="wp", bufs=2) as wpool,
            tc.tile_pool(name="ostage", bufs=6) as ostage,
            tc.tile_pool(name="mpsum", bufs=8, space="PSUM") as mpsum,
        ):
            wu = const.tile([P, 5 * P], bf16)
            nc.vector.memset(wu[:], 0.0)
            warm_ps = mpsum.tile([P, N_MM], f32, name="mmps")
            for _ in range(N_WARM):
                nc.tensor.matmul(
                    warm_ps[:], lhsT=wu[:, :P], rhs=wu[:, P:], start=True,
                    stop=True,
                )

            c128 = const.tile([P, 1], f32)
            nc.scalar.dma_start(out=c128[:], in_=coeff_d[:])

            xts = xpool.tile([P, K_T, CORE_M], bf16, name="xts")
            xts8 = xpool.tile([P, NPAIR, 2, CORE_M], f8, name="xts8")

            def x_slab(kt, eng, anchor=None):
                dx = eng.dma_start(
                    out=xts[:, kt, :], in_=xT_d[kt * P:(kt + 1) * P, :]
                )
                if anchor is not None:
                    tile.add_dep_helper(
                        dx.ins, anchor.ins, reason="pace x behind W chunk0"
                    )
                return dx

            def chunk_dma(np_, kt0, ch, stage_tag, base_eng=None, anchor=None):
                """DMA one base/mask chunk [kt0, kt0+ch) of panel np_."""
                rs = slice(np_ * P, (np_ + 1) * P)
                cs = slice(kt0 * N_MM, (kt0 + ch) * N_MM)
                bst = bstage.tile([P, ch, N_MM], bf16, name=f"bst{stage_tag}")
                mst = mstage.tile([P, ch, N_MM], i8, name=f"mst{stage_tag}")
                db = (base_eng or nc.sync).dma_start(out=bst[:], in_=base_d[rs, cs])
                dm = nc.scalar.dma_start(out=mst[:], in_=mask_d[rs, cs])
                if anchor is not None:
                    tile.add_dep_helper(db.ins, anchor.ins, reason="pace W chunk")
                    tile.add_dep_helper(dm.ins, anchor.ins, reason="pace W chunk")
                return bst, mst, db

            def chunk_fuse(wpb, wp8, kt0, ch, bst, mst):
                """Fuse W = base*S_W + (c*S_W)*mask; fp8 slabs to wp8."""
                stts = []
                for j in range(ch):
                    kt = kt0 + j
                    dst = (
                        wp8[:, kt // 2, kt % 2, :] if kt < F8
                        else wpb[:, kt - F8, :]
                    )
                    stts.append(nc.vector.scalar_tensor_tensor(
                        out=dst,
                        in0=mst[:, j, :],
                        scalar=c128[:, 0:1],
                        in1=bst[:, j, :],
                        op0=mybir.AluOpType.mult,
                        op1=mybir.AluOpType.add,
                    ))
                return stts

            def alloc_panel():
                wpb = wpool.tile([P, K_T - F8, N_MM], bf16, name="wp")
                wp8 = wpool.tile([P, NPAIR, 2, N_MM], f8, name="wp8")
                return wpb, wp8

            # Panel-0 prologue in consumption order: bf16 chunks (kts 4..31)
            # with x slabs interleaved, fp8 chunks (kts 0..3) and the fp8 x
            # slabs last — every panel consumes its fp8 pairs last.
            wp0 = alloc_panel()
            wp1 = alloc_panel()
            first_db = None
            p0_stts = []
            f8ch = F8 // 2
            for ci, c in enumerate(range(f8ch, K_T // 2)):
                bst, mst, db = chunk_dma(0, 2 * c, 2, "2")
                if first_db is None:
                    first_db = db
                anchor = first_db if ci >= 6 else None
                x_slab(2 * c, nc.gpsimd, anchor=anchor)
                x_slab(2 * c + 1, nc.sync, anchor=anchor)
                p0_stts += chunk_fuse(wp0[0], wp0[1], 2 * c, 2, bst, mst)
                if ci == 7:
                    # mid-prologue: panel 0's fp8 chunks and fp8 x slabs —
                    # consumed at the panel's end, but emitted here so they
                    # arrive ~25us before that deadline instead of just-in-
                    # time behind the whole bf16 stream.
                    for c8 in range(f8ch):
                        b8, m8, _ = chunk_dma(0, 2 * c8, 2, "2",
                                              anchor=first_db)
                        chunk_fuse(wp0[0], wp0[1], 2 * c8, 2, b8, m8)
                    for sl in range(F8):
                        dx = nc.gpsimd.dma_start(
                            out=xts8[:, sl // 2, sl % 2, :],
                            in_=xT8_d[sl * P:(sl + 1) * P, :],
                        )
                        tile.add_dep_helper(
                            dx.ins, first_db.ins,
                            reason="pace x8 behind W chunk0",
                        )

            # W1 chunks ride the ACT ring, dep-paced into panel 0's middle
            # so their bursts never displace panel-0-critical deliveries.
            # High-kt chunks are fetched/fused FIRST and panel 1
            # accumulates in REVERSE order to match the stream's pace.
            w1_stage = [
                (8 * q, chunk_dma(1, 8 * q, 8, "8", base_eng=nc.scalar,
                                  anchor=p0_stts[25 - 4 * q]))
                for q in reversed(range(4))
            ]
            for kt0, (bst, mst, _) in w1_stage:
                chunk_fuse(wp1[0], wp1[1], kt0, 8, bst, mst)

            def build_panel(np_, anchor=None):
                # anchor (current panel's last drain copy) keeps this
                # panel's chunk triggers from being scheduler-hoisted ahead
                # of the drain copies in the HWDGE engine queues.
                wpb, wp8 = alloc_panel()
                for c in range(4):
                    bst, mst, _ = chunk_dma(np_, 8 * c, 8, "8", anchor=anchor)
                    chunk_fuse(wpb, wp8, 8 * c, 8, bst, mst)
                return wpb, wp8

            wps = {0: wp0, 1: wp1}
            for np_ in range(NPAN):
                wpb, wp8 = wps.pop(np_)
                if np_ == 0:
                    waves = [(0, M_T)]
                elif np_ == NPAN - 1:
                    waves = [(0, 4), (4, 2), (6, 1), (7, 1)]
                else:
                    waves = [(0, 4), (4, 4)]
                # Panels 0/1: bf16 kts first (reversed for panel 1), fp8
                # pairs LAST (W-arrival deadlines). Panels 2+: pairs spread
                # evenly through the sequence so the 2x-power DoubleRow
                # bursts stay short and don't trip the P0 throttle.
                kts = list(range(F8, K_T))
                prs = list(range(NPAIR))
                if np_ == 1:
                    kts, prs = list(reversed(kts)), list(reversed(prs))
                if np_ >= 2:
                    seq = []
                    nb = len(kts)
                    pi = 0
                    for idx, kt in enumerate(kts):
                        seq.append(("b", kt))
                        if pi < NPAIR and (idx + 1) * NPAIR >= (pi + 1) * nb:
                            seq.append(("8", prs[pi]))
                            pi += 1
                    for pr in prs[pi:]:
                        seq.append(("8", pr))
                else:
                    seq = [("b", kt) for kt in kts] + [("8", pr) for pr in prs]
                for w0, wlen in waves:
                    psums = {
                        m: mpsum.tile([P, N_MM], f32, name="mmps")
                        for m in range(w0, w0 + wlen)
                    }
                    for si, (kind, idx) in enumerate(seq):
                        for m in range(w0, w0 + wlen):
                            if kind == "b":
                                nc.tensor.matmul(
                                    psums[m][:],
                                    lhsT=xts[:, idx, m * P:(m + 1) * P],
                                    rhs=wpb[:, idx - F8, :],
                                    start=(si == 0),
                                    stop=(si == len(seq) - 1),
                                )
                            else:
                                nc.tensor.matmul(
                                    psums[m][:],
                                    lhsT=xts8[:, idx, :, m * P:(m + 1) * P],
                                    rhs=wp8[:, idx, :, :],
                                    start=(si == 0),
                                    stop=(si == len(seq) - 1),
                                    perf_mode=mybir.MatmulPerfMode.DoubleRow,
                                )
                    for i, m in enumerate(sorted(psums)):
                        ob = ostage.tile([P, N_MM], f32, name="ob")
                        if np_ == 0 or i % 2 == 0:
                            last_cp = nc.scalar.mul(
                                out=ob[:], in_=psums[m][:], mul=DESCALE
                            )
                        else:
                            last_cp = nc.vector.tensor_scalar_mul(
                                out=ob[:], in0=psums[m][:], scalar1=DESCALE
                            )
                        col0 = np_ * N_MM
                        if wlen == 1:
                            h = N_MM // 2
                            nc.sync.dma_start(
                                out=out_d[m * P:(m + 1) * P, col0:col0 + h],
                                in_=ob[:, :h],
                            )
                            nc.scalar.dma_start(
                                out=out_d[
                                    m * P:(m + 1) * P, col0 + h:col0 + N_MM
                                ],
                                in_=ob[:, h:],
                            )
                        else:
                            dma_eng = nc.sync if i % 2 == 0 else nc.scalar
                            dma_eng.dma_start(
                                out=out_d[
                                    m * P:(m + 1) * P, col0:col0 + N_MM
                                ],
                                in_=ob[:],
                            )
                if np_ + 2 < NPAN:
                    wps[np_ + 2] = build_panel(np_ + 2, anchor=last_cp)

    nc.compile()
    return nc


_NC_CACHE = {}


def _get_nc():
    if "nc" not in _NC_CACHE:
        _NC_CACHE["nc"] = build_kernel()
    return _NC_CACHE["nc"]


def make_in_maps(x, base, coeff, mask):
    bf16 = ml_dtypes.bfloat16
    f8 = ml_dtypes.float8_e4m3
    x2 = np.asarray(x, np.float32).reshape(FULL_M, FULL_K)

    # W inputs pretiled to [NPAN, P, K_T, N_MM] (p-major panels) so each
    # partition's panel data is one contiguous DMA line. Shared by all 8
    # cores (W is replicated). base pre-scaled by SCALE_W (power of two;
    # exactly cancelled by the on-device PSUM descale).
    baseT = np.ascontiguousarray(
        (np.asarray(base, np.float32) * SCALE_W).astype(bf16)
        .reshape(K_T, P, NPAN, N_MM).transpose(2, 1, 0, 3)
    ).reshape(NPAN * P, K_T * N_MM)
    maskT = np.ascontiguousarray(
        np.asarray(mask).astype(np.int8)
        .reshape(K_T, P, NPAN, N_MM).transpose(2, 1, 0, 3)
    ).reshape(NPAN * P, K_T * N_MM)
    c128 = np.full(
        (P, 1), np.asarray(coeff, np.float32)[0] * SCALE_W, np.float32
    )

    in_maps = []
    for i in range(N_CORES):
        rows = slice(i * CORE_M, (i + 1) * CORE_M)
        xc = (x2[rows, :] * SCALE_X).astype(bf16)
        x8 = np.clip(x2[rows, :F8 * P] * SCALE_X, -240, 240).astype(f8)
        in_maps.append(
            {
                "xT": np.ascontiguousarray(xc.T),
                "xT8": np.ascontiguousarray(x8.T),
                "baseT": baseT,
                "maskT": maskT,
                "coeff": c128,
            }
        )
    return in_maps, x.shape[:2]


def assemble(results, B, L):
    out = np.concatenate([results[i]["out"] for i in range(N_CORES)], axis=0)
    return out.reshape(B, L, FULL_N)


def kernel(x, base, coeff, mask):
    from concourse.bass_utils import run_bass_kernel_spmd

    in_maps, (B, L) = make_in_maps(x, base, coeff, mask)
    nc = _get_nc()
    res = run_bass_kernel_spmd(nc, in_maps, list(range(8)))
    return assemble(res.results, B, L)


# revision 31
# speedup vs baseline: 1.0119x; 1.0119x over previous
"""Trainium2 Bass kernel for nn_BinaryDiff: out = x @ base + coeff * (x @ mask).

Fused as a single matmul: out = x @ W where W = base + coeff * mask.

Sharding over 8 NeuronCores: data-parallel over rows — each core computes
1024 rows x 4096 cols of the [8192, 4096] output (x sharded by rows, W
replicated). x is pre-transposed/cast on the host as part of the
sharding/layout prep, so the device program is a pure matmul pipeline.

Mixed precision: 28 of 32 k-slabs run bf16; k-slabs 0-3 run fp8(e4m3)
with perf_mode=DoubleRow (two k-slabs per matmul, ~2x throughput), which
keeps the measured rel-err ~1.4e-2 (sim-verified on the fixed inputs)
under the 2e-2 gate while cutting ~6% of PE time. Both paths share one
power-of-2 pre-scaling (x*16, W*1024 — keeps fp8 operands out of the
subnormal range) so they accumulate into the same PSUM bank at scale
2^14; the PSUM drain copies descale by 2^-14. The fp8 pairs are placed
LAST in every panel's accumulation so their operands have the most
relaxed arrival deadlines.

Pipeline structure:
  - x^T shard resident in SBUF, slab DMAs split across the SWDGE and SP
    rings (a single queue only gets ~1/3 of HBM bandwidth under
    round-robin) with the first slabs racing ahead to build run-ahead.
  - W streamed in eight 512-col panels (p-major host layout, contiguous
    per-partition lines), base chunks on the SP HWDGE ring, mask chunks
    on the ACT HWDGE ring, fused W = base*1024 + (c*1024)*mask on DVE
    into double-buffered panel caches (bf16 + fp8), one panel of
    prefetch lead. Panel np+2's chunk DMAs are anchored on panel np's
    last drain copy so the scheduler cannot hoist them ahead of the
    drains in the engine queues (that head-of-line-blocks the copies,
    pins PSUM banks, and stalls the next panel). W1 rides the ACT ring
    dep-paced into panel 0's middle, and panel 1 accumulates kt in
    REVERSE so its deadlines match W1's late high-kt arrivals.
  - Warm-up matmuls on a memset tile while the first chunks land, so the
    PE HAM clock-gate is already 8/8 when real work starts.
  - Panel 0 runs as one 8-bank wave (matmul consumption matches DMA
    delivery); later panels run 4-bank waves rotating through the 8 PSUM
    banks (drains overlap the next wave, no bank-handoff stalls); the
    last panel drains 4/2/1/1 with the final DMAs split across both
    HWDGE rings to shorten the end-of-kernel tail.
  - Panel-0 drains are ACT-only (a DVE copy would sit behind W1's fuses
    in the strict-FIFO DVE queue); later panels alternate ScalarE/DVE.
"""

import numpy as np
import ml_dtypes

import concourse.mybir as mybir
import concourse.tile as tile
from concourse import bacc

P = 128
FULL_M, FULL_K, FULL_N = 8192, 4096, 4096
N_CORES = 8
CORE_M = FULL_M // N_CORES      # 1024 rows per core
K_T = FULL_K // P               # 32 k-slabs
M_T = CORE_M // P               # 8 m-tiles
NPAN = 8                        # W panels across N
N_MM = FULL_N // NPAN           # 512 (one PSUM bank)
N_WARM = 26                     # HAM warm-up matmuls
# F8=6 with the 3 DoubleRow pairs back-to-back trips the chip's P0 power
# throttle (PE 2.4 -> 2.0 GHz for the whole run, +15% wall) — the fp8
# DoubleRow cells draw ~2x power. Spreading the pairs through the bf16
# sequence (panels 2+) keeps the burst short.
F8 = 6                          # leading k-slabs in fp8 DoubleRow
NPAIR = F8 // 2                 # DoubleRow pairs
SCALE_X = 16.0                  # fp8 subnormal-avoidance pre-scales
SCALE_W = 1024.0
DESCALE = 1.0 / (SCALE_X * SCALE_W)


def build_kernel(debug=False):
    """Build the per-core Bass program. All cores run the same program (SPMD)."""
    f32 = mybir.dt.float32
    i8 = mybir.dt.int8
    bf16 = mybir.dt.bfloat16
    f8 = mybir.dt.float8e4

    nc = bacc.Bacc("TRN2", target_bir_lowering=False, debug=debug)

    xT_d = nc.dram_tensor("xT", [FULL_K, CORE_M], bf16, kind="ExternalInput").ap()
    xT8_d = nc.dram_tensor("xT8", [F8 * P, CORE_M], f8, kind="ExternalInput").ap()
    # base/mask pretiled on host to [NPAN, P, K_T, N_MM] (p-major panels,
    # flattened to 2D) so per-partition DMA lines are contiguous.
    base_d = nc.dram_tensor(
        "baseT", [NPAN * P, K_T * N_MM], bf16, kind="ExternalInput"
    ).ap()
    mask_d = nc.dram_tensor(
        "maskT", [NPAN * P, K_T * N_MM], i8, kind="ExternalInput"
    ).ap()
    coeff_d = nc.dram_tensor("coeff", [P, 1], f32, kind="ExternalInput").ap()
    out_d = nc.dram_tensor("out", [CORE_M, FULL_N], f32, kind="ExternalOutput").ap()

    with tile.TileContext(nc) as tc:
        with (
            tc.tile_pool(name="const", bufs=1) as const,
            tc.tile_pool(name="xts", bufs=1) as xpool,
            tc.tile_pool(name="bstage", bufs=4) as bstage,
            tc.tile_pool(name="mstage", bufs=4) as mstage,
            tc.tile_pool(name="wp", bufs=2) as wpool,
            tc.tile_pool(name="ostage", bufs=6) as ostage,
            tc.tile_pool(name="mpsum", bufs=8, space="PSUM") as mpsum,
        ):
            wu = const.tile([P, 5 * P], bf16)
            nc.vector.memset(wu[:], 0.0)
            warm_ps = mpsum.tile([P, N_MM], f32, name="mmps")
            for _ in range(N_WARM):
                nc.tensor.matmul(
                    warm_ps[:], lhsT=wu[:, :P], rhs=wu[:, P:], start=True,
                    stop=True,
                )

            c128 = const.tile([P, 1], f32)
            nc.scalar.dma_start(out=c128[:], in_=coeff_d[:])

            xts = xpool.tile([P, K_T, CORE_M], bf16, name="xts")
            xts8 = xpool.tile([P, NPAIR, 2, CORE_M], f8, name="xts8")

            def x_slab(kt, eng, anchor=None):
                dx = eng.dma_start(
                    out=xts[:, kt, :], in_=xT_d[kt * P:(kt + 1) * P, :]
                )
                if anchor is not None:
                    tile.add_dep_helper(
                        dx.ins, anchor.ins, reason="pace x behind W chunk0"
                    )
                return dx

            def chunk_dma(np_, kt0, ch, stage_tag, base_eng=None, anchor=None):
                """DMA one base/mask chunk [kt0, kt0+ch) of panel np_."""
                rs = slice(np_ * P, (np_ + 1) * P)
                cs = slice(kt0 * N_MM, (kt0 + ch) * N_MM)
                bst = bstage.tile([P, ch, N_MM], bf16, name=f"bst{stage_tag}")
                mst = mstage.tile([P, ch, N_MM], i8, name=f"mst{stage_tag}")
                db = (base_eng or nc.sync).dma_start(out=bst[:], in_=base_d[rs, cs])
                dm = nc.scalar.dma_start(out=mst[:], in_=mask_d[rs, cs])
                if anchor is not None:
                    tile.add_dep_helper(db.ins, anchor.ins, reason="pace W chunk")
                    tile.add_dep_helper(dm.ins, anchor.ins, reason="pace W chunk")
                return bst, mst, db

            def chunk_fuse(wpb, wp8, kt0, ch, bst, mst):
                """Fuse W = base*S_W + (c*S_W)*mask; fp8 slabs to wp8."""
                stts = []
                for j in range(ch):
                    kt = kt0 + j
                    dst = (
                        wp8[:, kt // 2, kt % 2, :] if kt < F8
                        else wpb[:, kt - F8, :]
                    )
                    stts.append(nc.vector.scalar_tensor_tensor(
                        out=dst,
                        in0=mst[:, j, :],
                        scalar=c128[:, 0:1],
                        in1=bst[:, j, :],
                        op0=mybir.AluOpType.mult,
                        op1=mybir.AluOpType.add,
                    ))
                return stts

            def alloc_panel():
                wpb = wpool.tile([P, K_T - F8, N_MM], bf16, name="wp")
                wp8 = wpool.tile([P, NPAIR, 2, N_MM], f8, name="wp8")
                return wpb, wp8

            # Panel-0 prologue in consumption order: bf16 chunks (kts 4..31)
            # with x slabs interleaved, fp8 chunks (kts 0..3) and the fp8 x
            # slabs last — every panel consumes its fp8 pairs last.
            wp0 = alloc_panel()
            wp1 = alloc_panel()
            first_db = None
            p0_stts = []
            f8ch = F8 // 2
            for c in range(f8ch, K_T // 2):
                bst, mst, db = chunk_dma(0, 2 * c, 2, "2")
                if first_db is None:
                    first_db = db
                anchor = first_db if c >= f8ch + 4 else None
                x_slab(2 * c, nc.gpsimd, anchor=anchor)
                x_slab(2 * c + 1, nc.sync, anchor=anchor)
                p0_stts += chunk_fuse(wp0[0], wp0[1], 2 * c, 2, bst, mst)
            for c in range(f8ch):
                bst, mst, _ = chunk_dma(0, 2 * c, 2, "2", anchor=first_db)
                p0_stts += chunk_fuse(wp0[0], wp0[1], 2 * c, 2, bst, mst)
            for sl in range(F8):
                dx = nc.gpsimd.dma_start(
                    out=xts8[:, sl // 2, sl % 2, :],
                    in_=xT8_d[sl * P:(sl + 1) * P, :],
                )
                tile.add_dep_helper(
                    dx.ins, first_db.ins, reason="pace x8 behind W chunk0"
                )

            # W1 chunks ride the ACT ring, dep-paced into panel 0's middle
            # so their bursts never displace panel-0-critical deliveries.
            # High-kt chunks are fetched/fused FIRST and panel 1
            # accumulates in REVERSE order to match the stream's pace.
            w1_stage = [
                (8 * q, chunk_dma(1, 8 * q, 8, "8", base_eng=nc.scalar,
                                  anchor=p0_stts[25 - 4 * q]))
                for q in reversed(range(4))
            ]
            for kt0, (bst, mst, _) in w1_stage:
                chunk_fuse(wp1[0], wp1[1], kt0, 8, bst, mst)

            def build_panel(np_, anchor=None):
                # anchor (current panel's last drain copy) keeps this
                # panel's chunk triggers from being scheduler-hoisted ahead
                # of the drain copies in the HWDGE engine queues.
                wpb, wp8 = alloc_panel()
                for c in range(4):
                    bst, mst, _ = chunk_dma(np_, 8 * c, 8, "8", anchor=anchor)
                    chunk_fuse(wpb, wp8, 8 * c, 8, bst, mst)
                return wpb, wp8

            wps = {0: wp0, 1: wp1}
            for np_ in range(NPAN):
                wpb, wp8 = wps.pop(np_)
                if np_ == 0:
                    waves = [(0, M_T)]
                elif np_ == NPAN - 1:
                    waves = [(0, 4), (4, 2), (6, 1), (7, 1)]
                else:
                    waves = [(0, 4), (4, 4)]
                # Panels 0/1: bf16 kts first (reversed for panel 1), fp8
                # pairs LAST (W-arrival deadlines). Panels 2+: pairs spread
                # evenly through the sequence so the 2x-power DoubleRow
                # bursts stay short and don't trip the P0 throttle.
                kts = list(range(F8, K_T))
                prs = list(range(NPAIR))
                if np_ == 1:
                    kts, prs = list(reversed(kts)), list(reversed(prs))
                if np_ >= 2:
                    seq = []
                    nb = len(kts)
                    pi = 0
                    for idx, kt in enumerate(kts):
                        seq.append(("b", kt))
                        if pi < NPAIR and (idx + 1) * NPAIR >= (pi + 1) * nb:
                            seq.append(("8", prs[pi]))
                            pi += 1
                    for pr in prs[pi:]:
                        seq.append(("8", pr))
                else:
                    seq = [("b", kt) for kt in kts] + [("8", pr) for pr in prs]
                for w0, wlen in waves:
                    psums = {
                        m: mpsum.tile([P, N_MM], f32, name="mmps")
                        for m in range(w0, w0 + wlen)
                    }
                    for si, (kind, idx) in enumerate(seq):
                        for m in range(w0, w0 + wlen):
                            if kind == "b":
                                nc.tensor.matmul(
                                    psums[m][:],
                                    lhsT=xts[:, idx, m * P:(m + 1) * P],
                                    rhs=wpb[:, idx - F8, :],
                                    start=(si == 0),
                                    stop=(si == len(seq) - 1),
                                )
                            else:
                                nc.tensor.matmul(
                                    psums[m][:],
                                    lhsT=xts8[:, idx, :, m * P:(m + 1) * P],
                                    rhs=wp8[:, idx, :, :],
                                    start=(si == 0),
                                    stop=(si == len(seq) - 1),
                                    perf_mode=mybir.MatmulPerfMode.DoubleRow,
                                )
                    for i, m in enumerate(sorted(psums)):
                        ob = ostage.tile([P, N_MM], f32, name="ob")
                        if np_ == 0 or i % 2 == 0:
                            last_cp = nc.scalar.mul(
                                out=ob[:], in_=psums[m][:], mul=DESCALE
                            )
                        else:
                            last_cp = nc.vector.tensor_scalar_mul(
                                out=ob[:], in0=psums[m][:], scalar1=DESCALE
                            )
                        col0 = np_ * N_MM
                        if wlen == 1:
                            h = N_MM // 2
                            nc.sync.dma_start(
                                out=out_d[m * P:(m + 1) * P, col0:col0 + h],
                                in_=ob[:, :h],
                            )
                            nc.scalar.dma_start(
                                out=out_d[
                                    m * P:(m + 1) * P, col0 + h:col0 + N_MM
                                ],
                                in_=ob[:, h:],
                            )
                        else:
                            dma_eng = nc.sync if i % 2 == 0 else nc.scalar
                            dma_eng.dma_start(
                                out=out_d[
                                    m * P:(m + 1) * P, col0:col0 + N_MM
                                ],
                                in_=ob[:],
                            )
                if np_ + 2 < NPAN:
                    wps[np_ + 2] = build_panel(np_ + 2, anchor=last_cp)

    nc.compile()
    return nc


_NC_CACHE = {}


def _get_nc():
    if "nc" not in _NC_CACHE:
        _NC_CACHE["nc"] = build_kernel()
    return _NC_CACHE["nc"]


def make_in_maps(x, base, coeff, mask):
    bf16 = ml_dtypes.bfloat16
    f8 = ml_dtypes.float8_e4m3
    x2 = np.asarray(x, np.float32).reshape(FULL_M, FULL_K)

    # W inputs pretiled to [NPAN, P, K_T, N_MM] (p-major panels) so each
    # partition's panel data is one contiguous DMA line. Shared by all 8
    # cores (W is replicated). base pre-scaled by SCALE_W (power of two;
    # exactly cancelled by the on-device PSUM descale).
    baseT = np.ascontiguousarray(
        (np.asarray(base, np.float32) * SCALE_W).astype(bf16)
        .reshape(K_T, P, NPAN, N_MM).transpose(2, 1, 0, 3)
    ).reshape(NPAN * P, K_T * N_MM)
    maskT = np.ascontiguousarray(
        np.asarray(mask).astype(np.int8)
        .reshape(K_T, P, NPAN, N_MM).transpose(2, 1, 0, 3)
    ).reshape(NPAN * P, K_T * N_MM)
    c128 = np.full(
        (P, 1), np.asarray(coeff, np.float32)[0] * SCALE_W, np.float32
    )

    in_maps = []
    for i in range(N_CORES):
        rows = slice(i * CORE_M, (i + 1) * CORE_M)
        xc = (x2[rows, :] * SCALE_X).astype(bf16)
        x8 = np.clip(x2[rows, :F8 * P] * SCALE_X, -240, 240).astype(f8)
        in_maps.append(
            {
                "xT": np.ascontiguousarray(xc.T),
                "xT8": np.ascontiguousarray(x8.T),
                "baseT": baseT,
                "maskT": maskT,
                "coeff": c128,
            }
        )
    return in_maps, x.shape[:2]


def assemble(results, B, L):
    out = np.concatenate([results[i]["out"] for i in range(N_CORES)], axis=0)
    return out.reshape(B, L, FULL_N)


def kernel(x, base, coeff, mask):
    from concourse.bass_utils import run_bass_kernel_spmd

    in_maps, (B, L) = make_in_maps(x, base, coeff, mask)
    nc = _get_nc()
    res = run_bass_kernel_spmd(nc, in_maps, list(range(8)))
    return assemble(res.results, B, L)
